# revision 7
# baseline (speedup 1.0000x reference)
"""MLA attention layer on 8 trn2 NeuronCores.

Sharding: data-parallel over batch (2 groups of 4 cores) x tensor-parallel
over heads (4 heads per core). A-projections are replicated within each
batch group; B-projections, attention, and the Wo contraction are sharded
by head. Each core returns a partial [S, H] output for its batch; the host
sums the 4 partials per batch.
"""

import sys
import types
import numpy as np

import antenv  # noqa: F401

try:
    from antenv.axon_hooks import get_axon_ntff_profile_hook  # noqa: F401
except ImportError:
    from trn_agent_boot.trn_boot import _ntff_profile_via_ctypes

    _hooks_mod = types.ModuleType("antenv.axon_hooks")
    _HOOK = [_ntff_profile_via_ctypes("/opt/axon/libaxon_pjrt.so")]
    _hooks_mod.get_axon_ntff_profile_hook = lambda: _HOOK[0]
    _hooks_mod.set_axon_ntff_profile_hook = lambda h: _HOOK.__setitem__(0, h)
    sys.modules["antenv.axon_hooks"] = _hooks_mod

import ml_dtypes
import concourse.bacc as bacc
import concourse.tile as tile
from concourse import mybir
from concourse.bass_utils import run_bass_kernel_spmd

F32 = mybir.dt.float32
F32R = mybir.dt.float32r
BF16 = mybir.dt.bfloat16
AF = mybir.ActivationFunctionType
AX = mybir.AxisListType
ALU = mybir.AluOpType

EPS = 1e-5
NH = 16
DN = 128
DR = 64
DQK = DN + DR
DV = 128
KVR = 512
SCALE = DQK ** -0.5
S = 2048
H = 2048
QR = 1536
B = 2
NB = 8           # seq blocks of 256 in phase 1
BS = S // NB     # 256
QS = 512         # phase-2 q_super size
HPC = 4          # heads per core

_CACHED_NC = None


def _build():
    nc = bacc.Bacc("TRN2", target_bir_lowering=False)

    xT = nc.dram_tensor("xT", [H, S], F32R, kind="ExternalInput")
    wqa = nc.dram_tensor("wqa", [H, QR], F32R, kind="ExternalInput")
    wqb = nc.dram_tensor("wqb", [QR, 1024], F32R, kind="ExternalInput")
    wkva = nc.dram_tensor("wkva", [H, 640], F32R, kind="ExternalInput")
    wkvb = nc.dram_tensor("wkvb", [KVR, 1024], F32R, kind="ExternalInput")
    wo = nc.dram_tensor("wo", [HPC * DV, H], F32R, kind="ExternalInput")
    crep = nc.dram_tensor("crep", [128, S], F32, kind="ExternalInput")
    srep = nc.dram_tensor("srep", [128, S], F32, kind="ExternalInput")
    ropeA = nc.dram_tensor("ropeA", [128, 128], F32R, kind="ExternalInput")
    ropeB = nc.dram_tensor("ropeB", [128, 128], F32R, kind="ExternalInput")
    ones128 = nc.dram_tensor("ones128", [128, 128], F32R, kind="ExternalInput")
    identb = nc.dram_tensor("identb", [128, 128], BF16, kind="ExternalInput")
    cmask = nc.dram_tensor("cmask", [128, 128], F32, kind="ExternalInput")
    outp = nc.dram_tensor("outp", [S, H], F32, kind="ExternalOutput")
    # spill buffer for q^T (nope 4 tiles + rope 4 tiles)
    qspill = nc.dram_tensor("qspill", [8, 128, S], F32R, kind="Internal")

    with tile.TileContext(nc) as tc:
        with (
            tc.tile_pool(name="persist", bufs=1) as pp,
            tc.tile_pool(name="consts", bufs=1) as cp,
        ):
            # persistent K/V state
            knope_sb = pp.tile([128, HPC, S], F32R, tag="knope")
            krope_sb = pp.tile([128, S], F32R, tag="krope")
            v_sb = pp.tile([128, 16, 512], BF16, tag="v")

            crep_sb = cp.tile([128, S], F32, tag="crep")
            srep_sb = cp.tile([128, S], F32, tag="srep")
            ropeA_sb = cp.tile([128, 128], F32R, tag="ropeA")
            ropeB_sb = cp.tile([128, 128], F32R, tag="ropeB")
            ones_sb = cp.tile([128, 128], F32R, tag="ones")
            ident_sb = cp.tile([128, 128], BF16, tag="ident")
            cmask_sb = cp.tile([128, 128], F32, tag="cmask")
            nc.sync.dma_start(out=crep_sb, in_=crep[:, :])
            nc.sync.dma_start(out=srep_sb, in_=srep[:, :])
            nc.sync.dma_start(out=ropeA_sb, in_=ropeA[:, :])
            nc.sync.dma_start(out=ropeB_sb, in_=ropeB[:, :])
            nc.sync.dma_start(out=ones_sb, in_=ones128[:, :])
            nc.sync.dma_start(out=ident_sb, in_=identb[:, :])
            nc.sync.dma_start(out=cmask_sb, in_=cmask[:, :])
            eps_sb = cp.tile([128, 1], F32, tag="eps")
            nc.vector.memset(eps_sb, EPS)

            # ---------------- phase 1: projections ----------------
            with (
                tc.tile_pool(name="p1", bufs=1) as p1,
                tc.tile_pool(name="p1w", bufs=2) as p1w,
                tc.tile_pool(name="p1t", bufs=3) as p1t,
                tc.tile_pool(name="ps1", bufs=3, space="PSUM") as ps1,
                tc.tile_pool(name="ps1r", bufs=1, space="PSUM") as ps1r,
            ):
                wtv = p1.tile([128, 4, 512], F32R, tag="wkvbv")
                nc.sync.dma_start(
                    out=wtv,
                    in_=wkvb[:, 512:1024].rearrange("(k p) c -> p k c", p=128))
                for blk in range(NB):
                    sl = slice(blk * BS, (blk + 1) * BS)
                    xt = p1.tile([128, 16, BS], F32R, tag="xt")
                    nc.sync.dma_start(
                        out=xt, in_=xT[:, sl].rearrange("(k p) s -> p k s", p=128))

                    # ---- qa^T = (x Wqa)^T : [1536, BS] as 12 tiles
                    qa = p1.tile([128, 12, BS], F32R, tag="qa")
                    sqacc = ps1r.tile([128, BS], F32, tag="sqacc")
                    for m in range(12):
                        wt = p1w.tile([128, 16, 128], F32R, tag="wqa")
                        nc.sync.dma_start(
                            out=wt,
                            in_=wqa[:, m * 128:(m + 1) * 128].rearrange(
                                "(k p) c -> p k c", p=128))
                        ps = ps1.tile([128, BS], F32, tag="mm")
                        for k in range(16):
                            nc.tensor.matmul(ps[:, :], wt[:, k, :], xt[:, k, :],
                                             start=(k == 0), stop=(k == 15))
                        nc.scalar.copy(qa[:, m, :], ps[:, :])
                        # squares for RMS
                        sq = p1t.tile([128, BS], F32R, tag="sq")
                        nc.vector.tensor_mul(sq, qa[:, m, :].bitcast(F32),
                                             qa[:, m, :].bitcast(F32))
                        nc.tensor.matmul(sqacc[:, :], ones_sb[:, :], sq,
                                         start=(m == 0), stop=(m == 11))
                    # rstd_q broadcast [128, BS]
                    rstdq = p1t.tile([128, BS], F32, tag="rstdq")
                    nc.scalar.activation(rstdq, sqacc[:, :], AF.Sqrt,
                                         bias=eps_sb[:, :], scale=1.0 / QR)
                    nc.vector.reciprocal(rstdq, rstdq)
                    for m in range(12):
                        nc.vector.tensor_mul(qa[:, m, :],
                                             qa[:, m, :].bitcast(F32), rstdq)

                    # ---- kva^T = (x Wkva)^T : 5 tiles (4 latent + rope/zero)
                    kva = p1.tile([128, 5, BS], F32R, tag="kva")
                    sqacck = ps1r.tile([128, BS], F32, tag="sqacck")
                    for m in range(5):
                        wt = p1w.tile([128, 16, 128], F32R, tag="wkva")
                        nc.sync.dma_start(
                            out=wt,
                            in_=wkva[:, m * 128:(m + 1) * 128].rearrange(
                                "(k p) c -> p k c", p=128))
                        ps = ps1.tile([128, BS], F32, tag="mm")
                        for k in range(16):
                            nc.tensor.matmul(ps[:, :], wt[:, k, :], xt[:, k, :],
                                             start=(k == 0), stop=(k == 15))
                        nc.scalar.copy(kva[:, m, :], ps[:, :])
                        if m < 4:
                            sq = p1t.tile([128, BS], F32R, tag="sq")
                            nc.vector.tensor_mul(sq, kva[:, m, :].bitcast(F32),
                                                 kva[:, m, :].bitcast(F32))
                            nc.tensor.matmul(sqacck[:, :], ones_sb[:, :], sq,
                                             start=(m == 0), stop=(m == 3))
                    rstdk = p1t.tile([128, BS], F32, tag="rstdk")
                    nc.scalar.activation(rstdk, sqacck[:, :], AF.Sqrt,
                                         bias=eps_sb[:, :], scale=1.0 / KVR)
                    nc.vector.reciprocal(rstdk, rstdk)
                    for m in range(4):
                        nc.vector.tensor_mul(kva[:, m, :],
                                             kva[:, m, :].bitcast(F32), rstdk)

                    # ---- k_rope rotation (rows 64.. of kva[:,4,:] are zero)
                    ra = p1t.tile([128, BS], F32R, tag="ra")
                    rb = p1t.tile([128, BS], F32R, tag="rb")
                    nc.vector.tensor_mul(ra, kva[:, 4, :].bitcast(F32),
                                         crep_sb[:, sl])
                    nc.vector.tensor_mul(rb, kva[:, 4, :].bitcast(F32),
                                         srep_sb[:, sl])
                    psr = ps1.tile([128, BS], F32, tag="mm")
                    nc.tensor.matmul(psr[:, :], ropeA_sb[:, :], ra,
                                     start=True, stop=False)
                    nc.tensor.matmul(psr[:, :], ropeB_sb[:, :], rb,
                                     start=False, stop=True)
                    nc.scalar.copy(krope_sb[:, sl], psr[:, :])

                    # ---- q^T = (qa_norm Wqb)^T : 4 nope + 4 rope(padded)
                    for m in range(8):
                        wt = p1w.tile([128, 12, 128], F32R, tag="wqb")
                        nc.sync.dma_start(
                            out=wt,
                            in_=wqb[:, m * 128:(m + 1) * 128].rearrange(
                                "(k p) c -> p k c", p=128))
                        ps = ps1.tile([128, BS], F32, tag="mm")
                        for k in range(12):
                            nc.tensor.matmul(ps[:, :], wt[:, k, :], qa[:, k, :],
                                             start=(k == 0), stop=(k == 11))
                        if m < 4:
                            qn = p1t.tile([128, BS], F32R, tag="qn")
                            nc.scalar.copy(qn, ps[:, :])
                            nc.sync.dma_start(out=qspill[m, :, sl], in_=qn)
                        else:
                            ra = p1t.tile([128, BS], F32R, tag="ra")
                            rb = p1t.tile([128, BS], F32R, tag="rb")
                            nc.vector.tensor_mul(ra, ps[:, :], crep_sb[:, sl])
                            nc.vector.tensor_mul(rb, ps[:, :], srep_sb[:, sl])
                            ps2 = ps1.tile([128, BS], F32, tag="mm2")
                            nc.tensor.matmul(ps2[:, :], ropeA_sb[:, :], ra,
                                             start=True, stop=False)
                            nc.tensor.matmul(ps2[:, :], ropeB_sb[:, :], rb,
                                             start=False, stop=True)
                            qn = p1t.tile([128, BS], F32R, tag="qn")
                            nc.scalar.copy(qn, ps2[:, :])
                            nc.sync.dma_start(out=qspill[m, :, sl], in_=qn)

                    # ---- kv_b: k_nope^T per head + v natural
                    for m in range(4):
                        wt = p1w.tile([128, 4, 128], F32R, tag="wkvbn")
                        nc.sync.dma_start(
                            out=wt,
                            in_=wkvb[:, m * 128:(m + 1) * 128].rearrange(
                                "(k p) c -> p k c", p=128))
                        ps = ps1.tile([128, BS], F32, tag="mm")
                        for k in range(4):
                            nc.tensor.matmul(ps[:, :], wt[:, k, :], kva[:, k, :],
                                             start=(k == 0), stop=(k == 3))
                        nc.scalar.copy(knope_sb[:, m, sl], ps[:, :])
                    # v natural: [seq128 x 512] per seq sub-chunk
                    for sc in range(BS // 128):
                        g = (blk * BS) // 128 + sc
                        ps = ps1.tile([128, 512], F32, tag="mm")
                        for k in range(4):
                            nc.tensor.matmul(
                                ps[:, :], kva[:, k, sc * 128:(sc + 1) * 128],
                                wtv[:, k, :], start=(k == 0), stop=(k == 3))
                        nc.scalar.copy(v_sb[:, g, :], ps[:, :])

            # ---------------- phase 2: attention + Wo ----------------
            with (
                tc.tile_pool(name="p2w", bufs=1) as p2w,
                tc.tile_pool(name="p2q", bufs=3) as p2q,
                tc.tile_pool(name="p2p", bufs=8) as p2p,
                tc.tile_pool(name="p2pt", bufs=20) as p2pt,
                tc.tile_pool(name="p2s", bufs=6) as p2s,
                tc.tile_pool(name="p2a", bufs=6) as p2a,
                tc.tile_pool(name="p2o", bufs=3) as p2o,
                tc.tile_pool(name="ps2s", bufs=4, space="PSUM") as ps2s,
                tc.tile_pool(name="ps2t", bufs=2, space="PSUM") as ps2t,
                tc.tile_pool(name="ps2a", bufs=1, space="PSUM") as ps2a,
                tc.tile_pool(name="ps2o", bufs=1, space="PSUM") as ps2o,
            ):
                wo_sb = p2w.tile([128, HPC, H], F32R, tag="wo")
                nc.sync.dma_start(
                    out=wo_sb, in_=wo.rearrange("(g p) n -> p g n", p=128))
                for qs in range(4):
                    attn = {}
                    for h in range(HPC):
                        qsl = slice(qs * QS, (qs + 1) * QS)
                        qtn = p2q.tile([128, QS], F32R, tag="qtn")
                        qtr = p2q.tile([128, QS], F32R, tag="qtr")
                        nc.sync.dma_start(out=qtn, in_=qspill[h, :, qsl])
                        nc.sync.dma_start(out=qtr, in_=qspill[4 + h, :, qsl])

                        nchunk = qs + 1
                        pts = []
                        for g in range(4 * qs + 4):
                            pt = p2pt.tile([128, QS], BF16, tag="PT")
                            if g >= 4 * qs:
                                nc.gpsimd.memset(pt, 0.0)
                            pts.append(pt)

                        for i in range(4):
                            widths = [QS] * qs + [128 * (i + 1)]
                            qcol = slice(i * 128, (i + 1) * 128)
                            sps = []
                            mxs = p2s.tile([128, 8], F32, tag="mxs")
                            sums = p2s.tile([128, 8], F32, tag="sums")
                            for kc in range(nchunk):
                                w = widths[kc]
                                ksl = slice(kc * QS, kc * QS + w)
                                ps = ps2s.tile([128, QS], F32, tag="sc")
                                nc.tensor.matmul(ps[:, :w], qtn[:, qcol],
                                                 knope_sb[:, h, ksl],
                                                 start=True, stop=False)
                                nc.tensor.matmul(ps[:, :w], qtr[:, qcol],
                                                 krope_sb[:, ksl],
                                                 start=False, stop=True)
                                if kc == qs:
                                    nc.vector.tensor_add(
                                        ps[:, w - 128:w], ps[:, w - 128:w],
                                        cmask_sb)
                                nc.vector.tensor_reduce(
                                    mxs[:, kc:kc + 1], ps[:, :w],
                                    axis=AX.X, op=ALU.max)
                                sps.append(ps)
                            nmx = p2s.tile([128, 1], F32, tag="nmx")
                            nc.vector.tensor_reduce(nmx, mxs[:, :nchunk],
                                                    axis=AX.X, op=ALU.max,
                                                    negate=True)
                            pcs = []
                            for kc in range(nchunk):
                                w = widths[kc]
                                pc = p2p.tile([128, QS], BF16, tag="p")
                                nc.scalar.activation(
                                    pc[:, :w], sps[kc][:, :w], AF.Exp,
                                    bias=nmx, scale=1.0,
                                    accum_out=sums[:, kc:kc + 1])
                                pcs.append(pc)
                            ssum = p2s.tile([128, 1], F32, tag="ssum")
                            nc.vector.tensor_reduce(ssum, sums[:, :nchunk],
                                                    axis=AX.X, op=ALU.add)
                            rinv = p2s.tile([128, 1], F32, tag="rinv")
                            nc.vector.reciprocal(rinv, ssum)
                            for kc in range(nchunk):
                                w = widths[kc]
                                nc.vector.tensor_scalar_mul(
                                    pcs[kc][:, :w], pcs[kc][:, :w], rinv)
                                for j in range(w // 128):
                                    g = kc * 4 + j
                                    pt_ps = ps2t.tile([128, 128], BF16,
                                                      tag="pt")
                                    nc.tensor.transpose(
                                        pt_ps,
                                        pcs[kc][:, j * 128:(j + 1) * 128],
                                        ident_sb)
                                    nc.scalar.copy(pts[g][:, qcol], pt_ps)

                        # pv: out_attnT [dv, 512q]
                        psa = ps2a.tile([128, QS], F32, tag="pv")
                        ng = 4 * qs + 4
                        for g in range(ng):
                            nc.tensor.matmul(
                                psa[:, :], v_sb[:, g, h * 128:(h + 1) * 128],
                                pts[g][:, :], start=(g == 0), stop=(g == ng - 1))
                        at = p2a.tile([128, QS], F32R, tag="attnT")
                        nc.scalar.copy(at, psa[:, :])
                        attn[h] = at

                    # Wo for this q_super
                    for i in range(4):
                        orow = p2o.tile([128, H], F32, tag="orow")
                        for nch in range(4):
                            pso = ps2o.tile([128, 512], F32, tag="wo")
                            for h in range(HPC):
                                nc.tensor.matmul(
                                    pso[:, :], attn[h][:, i * 128:(i + 1) * 128],
                                    wo_sb[:, h, nch * 512:(nch + 1) * 512],
                                    start=(h == 0), stop=(h == 3))
                            nc.scalar.copy(orow[:, nch * 512:(nch + 1) * 512],
                                           pso[:, :])
                        nc.sync.dma_start(
                            out=outp[qs * QS + i * 128:qs * QS + (i + 1) * 128, :],
                            in_=orow)

    nc.finalize()
    return nc


def _get_nc():
    global _CACHED_NC
    if _CACHED_NC is None:
        _CACHED_NC = _build()
    return _CACHED_NC


def prepare_inputs(x, positions, Wqa, g_qa, Wqb, Wkva, g_kva, Wkvb, Wo,
                   cos, sin):
    x = np.asarray(x, dtype=np.float32)
    positions = np.asarray(positions)
    Wqa = np.asarray(Wqa, dtype=np.float32)
    g_qa = np.asarray(g_qa, dtype=np.float32)
    Wqb = np.asarray(Wqb, dtype=np.float32)
    Wkva = np.asarray(Wkva, dtype=np.float32)
    g_kva = np.asarray(g_kva, dtype=np.float32)
    Wkvb = np.asarray(Wkvb, dtype=np.float32)
    Wo = np.asarray(Wo, dtype=np.float32)
    cos = np.asarray(cos, dtype=np.float32)
    sin = np.asarray(sin, dtype=np.float32)

    c = cos[positions]  # [S, 32]
    s = sin[positions]
    crep = np.ascontiguousarray(np.tile(c.T, (4, 1)))  # [128, S]
    srep = np.ascontiguousarray(np.tile(s.T, (4, 1)))

    ropeA = np.zeros((128, 128), dtype=np.float32)
    ropeA[:64, :64] = np.eye(64)
    ropeB = np.zeros((128, 128), dtype=np.float32)
    ropeB[0:32, 32:64] = np.eye(32)
    ropeB[32:64, 0:32] = -np.eye(32)

    ones128 = np.ones((128, 128), dtype=np.float32)
    identb = np.eye(128, dtype=np.float32).astype(ml_dtypes.bfloat16)
    cmask = np.where(np.arange(128)[:, None] >= np.arange(128)[None, :],
                     0.0, -1e9).astype(np.float32)

    wqb_g = Wqb * g_qa[:, None] * SCALE  # [QR, NH*DQK]
    wqb_r = wqb_g.reshape(QR, NH, DQK)
    wkvb_g = (Wkvb * g_kva[:, None]).reshape(KVR, NH, DN + DV)
    wo_r = Wo.reshape(NH, DV, H)
    wkva_pad = np.zeros((H, 640), dtype=np.float32)
    wkva_pad[:, :576] = Wkva

    in_maps = []
    for core in range(8):
        b = core // 4
        hg = core % 4
        heads = slice(hg * HPC, (hg + 1) * HPC)
        nope = wqb_r[:, heads, :DN].reshape(QR, HPC * DN)
        rope = wqb_r[:, heads, DN:]  # [QR, 4, 64]
        rope_pad = np.zeros((QR, HPC, 128), dtype=np.float32)
        rope_pad[:, :, :64] = rope
        wqb_c = np.concatenate([nope, rope_pad.reshape(QR, HPC * 128)], axis=1)
        knope = wkvb_g[:, heads, :DN].reshape(KVR, HPC * DN)
        vpart = wkvb_g[:, heads, DN:].reshape(KVR, HPC * DV)
        wkvb_c = np.concatenate([knope, vpart], axis=1)
        wo_c = wo_r[heads].reshape(HPC * DV, H)
        in_maps.append({
            "xT": np.ascontiguousarray(x[b].T),
            "wqa": Wqa,
            "wqb": np.ascontiguousarray(wqb_c),
            "wkva": wkva_pad,
            "wkvb": np.ascontiguousarray(wkvb_c),
            "wo": np.ascontiguousarray(wo_c),
            "crep": crep,
            "srep": srep,
            "ropeA": ropeA,
            "ropeB": ropeB,
            "ones128": ones128,
            "identb": identb,
            "cmask": cmask,
        })
    return in_maps


def run(in_maps, trace=False):
    nc = _get_nc()
    return run_bass_kernel_spmd(nc, in_maps, core_ids=list(range(8)),
                                trace=trace)


def kernel(**inputs) -> np.ndarray:
    in_maps = prepare_inputs(**inputs)
    res = run(in_maps)
    out = np.zeros((B, S, H), dtype=np.float32)
    for core in range(8):
        out[core // 4] += res.results[core]["outp"]
    return out
